# revision 45
# baseline (speedup 1.0000x reference)
"""Trainium2 Bass kernel for nn_Attention_3710851743764.

Full attention block: qkv proj -> per-head RMSNorm(q,k) -> RoPE -> GQA
attention (16 q heads, 4 kv heads, S=2048, D=128) -> out proj.

Sharding: 8 cores = 2 (batch) x 4 (kv-head groups). Each core computes its
batch's qkv for its group (4 q heads + 1 kv head), full attention for those
heads, and a partial output projection (its 512 wo columns); the host sums
the 4 partials per batch.

Projection runs in f32r (full PE rate, no input quantization); attention
data (q/k/v, probabilities, outputs) is bf16 with f32 PSUM accumulation.
Dataflow is fully "transposed" (features on partitions, tokens on free):
  qkvT[f,t]   = mm(lhsT=wqkvT[d,f], rhs=xT[d,t])   kc-outer: 6 psum accums
  ssq[c,t]    = mm(lhsT=esel[:,c,:], rhs=square(qkvT_c))     (RMS factors)
  qn          = qkv * normw * rfac  (stt on DVE, emitted inside stage A)
  rot         = mm(lhsT=P_rot, rhs=qn)   (k and q0 in A's tail, q1-3
                tucked into D-block tails so the in-order PE never stalls)
  scoresT[s,t]= mm(lhsT=kT[:,s-blk], rhs=qT_h)    per 128-s block
  pT          = exp(scoresT)       (no max subtraction: |score|<=sqrt(128))
  attnT[d,t]  = mm(lhsT=v[s-blk,d], rhs=pT)       accumulated over s
  denom       = DVE bf16 accumulation of pT over s-blocks + tiny esel
                matmuls, accumulated on SBUF (frees PSUM for E interleave)
  out[t,o]    = mm(lhsT=attnT_n[f,t-blk], rhs=woT[f,o]); the first half
                of E runs inside pair-1's attention (PE slack under exp)
"""

import sys

sys.path.insert(0, "/opt/trn_rl_repo")

import numpy as np
import ml_dtypes

import concourse.bass as bass
import concourse.tile as tile
from concourse import bacc, mybir
from concourse import bass_utils

F32 = mybir.dt.float32
F32R = mybir.dt.float32r
BF16 = mybir.dt.bfloat16
AF = mybir.ActivationFunctionType
OP = mybir.AluOpType

DIM = 2048
N_HEADS = 16
N_KV = 4
HEAD_DIM = 128
B = 2
S = 2048
EPS = float(np.finfo(np.float32).eps)
GQ = N_HEADS // N_KV          # q heads per group = 4
GF = GQ * HEAD_DIM            # group q features = 512
P = 128
KC = DIM // P                 # 16 contraction chunks for projections
TC = 4                        # token chunks of 512
SC = S // P                   # 16 key chunks of 128
NF = GF + 2 * HEAD_DIM        # 768 qkv features per group
FC = NF // P                  # 6 feature chunks

_CACHED_NC = None


def build_nc():
    """Build the single-core Bass program (same program for all 8 cores)."""
    nc = bacc.Bacc("TRN2", target_bir_lowering=False, debug=False,
                   num_devices=8)

    xT_d = nc.dram_tensor("xT", [TC, P, KC, 512], F32R,
                          kind="ExternalInput").ap()
    wqkvT_d = nc.dram_tensor("wqkvT", [P, KC, NF], F32R,
                             kind="ExternalInput").ap()
    woT_d = nc.dram_tensor("woT", [HEAD_DIM, GQ, DIM], BF16,
                           kind="ExternalInput").ap()
    cosT_d = nc.dram_tensor("cosT", [HEAD_DIM, S], BF16,
                            kind="ExternalInput").ap()
    sinT_d = nc.dram_tensor("sinT", [HEAD_DIM, S], BF16,
                            kind="ExternalInput").ap()
    normw_d = nc.dram_tensor("normw", [P, 2], F32, kind="ExternalInput").ap()
    prot_d = nc.dram_tensor("prot", [P, P], BF16, kind="ExternalInput").ap()
    ident_d = nc.dram_tensor("ident", [P, P], BF16, kind="ExternalInput").ap()
    esel_d = nc.dram_tensor("esel", [P, 5, 5], BF16,
                            kind="ExternalInput").ap()
    out_d = nc.dram_tensor("out", [SC, P, DIM], BF16,
                           kind="ExternalOutput").ap()

    with tile.TileContext(nc) as tc:
        with (
            tc.tile_pool(name="consts", bufs=1) as cp,
        ):
            dramp = tc.alloc_tile_pool(name="dram_scratch", bufs=1,
                                       space="DRAM")
            rfac_dr = dramp.tile([5, S], F32, name="rfac_dr")
            rd_dr = [dramp.tile([4, 1024], BF16, name=f"rd_dr{i}")
                     for i in range(2)]
            # right-side stack: p2 (lives A..D) below p1 (lives A..mid-D)
            p2 = tc.alloc_tile_pool(name="p2", bufs=1, side="right")
            qk_sb = [p2.tile([P, S], BF16, name=f"qk_sb{i}")
                     for i in range(5)]                           # 20KB
            v_sb = p2.tile([P, SC, HEAD_DIM], BF16, name="v_sb")  # 4KB
            p1 = tc.alloc_tile_pool(name="p1", bufs=1, side="right")
            qkv_raw = p1.tile([P, 5, S], BF16, name="qkv_raw")    # 20KB
            vT_sb = p1.tile([P, S], BF16, name="vT_sb")           # 4KB
            rfac = p1.tile([5, S], F32, name="rfac")
            qn_sb = [p1.tile([P, S], BF16, name=f"qn_sb{i}")
                     for i in range(5)]                           # 20KB

            cos_sb = cp.tile([HEAD_DIM, S], BF16, name="cos_sb")
            sin_sb = cp.tile([HEAD_DIM, S], BF16, name="sin_sb")
            normw_sb = cp.tile([P, 2], F32, name="normw_sb")
            prot_sb = cp.tile([P, P], BF16, name="prot_sb")
            ident_sb = cp.tile([P, P], BF16, name="ident_sb")
            esel_sb = cp.tile([P, 5, 5], BF16, name="esel_sb")
            eps_sb = cp.tile([P, 1], F32, name="eps_sb")
            zero_sb = cp.tile([P, 1], F32, name="zero_sb")
            nc.vector.memset(eps_sb[:], EPS)
            nc.vector.memset(zero_sb[:], 0.0)
            # consts via the gpsimd queue; sync queue feeds stage A
            nc.gpsimd.dma_start(esel_sb[:], esel_d)
            nc.gpsimd.dma_start(ident_sb[:], ident_d)
            nc.gpsimd.dma_start(prot_sb[:], prot_d)
            nc.gpsimd.dma_start(normw_sb[:], normw_d)
            nc.gpsimd.dma_start(cos_sb[:], cosT_d)
            nc.gpsimd.dma_start(sin_sb[:], sinT_d)

            ropep = tc.alloc_tile_pool(name="ropep", bufs=1)

            def stt_chunk(fc, tcc):
                """qn = qkv * normw * rfac for one (feature, token) chunk."""
                tsl = slice(tcc * 512, (tcc + 1) * 512)
                wcol = 0 if fc < 4 else 1
                rb = ropep.tile([P, 512], F32, name="rb", tag="rb", bufs=3)
                nc.gpsimd.dma_start(
                    rb[:],
                    rfac_dr[fc:fc + 1, tsl].to_broadcast((P, 512)))
                nc.vector.scalar_tensor_tensor(
                    qn_sb[fc][:, tsl], qkv_raw[:, fc, tsl],
                    normw_sb[:, wcol:wcol + 1], rb[:],
                    op0=OP.mult, op1=OP.mult)

            rot_ps = {}

            def rope_rots(fc, pool, tag):
                """The four [P,512] rotation matmuls for one feature row."""
                rot_ps[fc] = []
                for tcc in range(TC):
                    rp = pool.tile([P, 512], F32, name="rot_ps", tag=tag)
                    nc.tensor.matmul(
                        rp[:], prot_sb[:],
                        qn_sb[fc][:, tcc * 512:(tcc + 1) * 512],
                        start=True, stop=True)
                    rot_ps[fc].append(rp)

            def rope_finish_chunk(fc, tcc, rp):
                """qk = qn*cos (Pool) + rot*sin (DVE), added in place."""
                tsl = slice(tcc * 512, (tcc + 1) * 512)
                nc.gpsimd.tensor_mul(qk_sb[fc][:, tsl],
                                     qn_sb[fc][:, tsl], cos_sb[:, tsl])
                rs = ropep.tile([P, 512], BF16, name="rs", tag="rs",
                                bufs=3)
                nc.vector.tensor_mul(rs[:], rp[:], sin_sb[:, tsl])
                nc.vector.tensor_add(qk_sb[fc][:, tsl],
                                     qk_sb[fc][:, tsl], rs[:])

            def rope_finish(fc):
                for tcc in range(TC):
                    rope_finish_chunk(fc, tcc, rot_ps[fc][tcc])

            def rope_chunk_inline(fc, tcc, pool, tag):
                """stt + rot + finish for one (feature, token) chunk."""
                stt_chunk(fc, tcc)
                rp = pool.tile([P, 512], F32, name="rot_ps", tag=tag)
                nc.tensor.matmul(
                    rp[:], prot_sb[:],
                    qn_sb[fc][:, tcc * 512:(tcc + 1) * 512],
                    start=True, stop=True)
                rope_finish_chunk(fc, tcc, rp)

            # ---------------- Stage A: qkv projection + squares ----------
            # kc-outer so the first matmuls only need the first x/w chunks
            # (fast DMA ramp); 6 psum accumulators live per token chunk.
            # The rmsnorm stt chunks ride along at each token-chunk tail,
            # and the k/q0 ropes run at the very end (rots via psSq ring).
            with (
                tc.tile_pool(name="stA", bufs=4) as sa,
                tc.tile_pool(name="sqp", bufs=2) as sqp,
                tc.tile_pool(name="wq_pool", bufs=1) as wp,
                tc.tile_pool(name="psA", bufs=1, space="PSUM") as psA,
                tc.tile_pool(name="psSq", bufs=2, space="PSUM") as psSq,
            ):
                wq_sb = wp.tile([P, KC, NF], F32R, name="wq_sb")  # 48KB
                for tcc in range(TC):
                    xts = []
                    for kc4 in range(0, KC, 4):
                        if tcc == 0:
                            nc.sync.dma_start(wq_sb[:, kc4:kc4 + 4, :],
                                              wqkvT_d[:, kc4:kc4 + 4, :])
                        xtc = sa.tile([P, 4, 512], F32R, name="xt",
                                      tag="xt")
                        nc.sync.dma_start(xtc[:],
                                          xT_d[tcc, :, kc4:kc4 + 4, :])
                        xts.append(xtc)
                    accs = [psA.tile([P, 512], F32, name=f"acc{fc}")
                            for fc in range(FC)]
                    for kc in range(KC):
                        for fc in range(FC):
                            nc.tensor.matmul(
                                accs[fc][:],
                                wq_sb[:, kc, fc * P:(fc + 1) * P],
                                xts[kc // 4][:, kc % 4, :],
                                start=(kc == 0), stop=(kc == KC - 1))
                    tsl = slice(tcc * 512, (tcc + 1) * 512)
                    ssq_ps = psSq.tile([5, 512], F32, name="ssq_ps",
                                       tag="sq")
                    for fc in range(FC):
                        if fc < 5:
                            sq = sqp.tile([P, 512], BF16, name="sq")
                            nc.scalar.activation(sq[:], accs[fc][:],
                                                 AF.Square, bias=zero_sb[:])
                            nc.tensor.matmul(
                                ssq_ps[:], esel_sb[:, fc, 0:5], sq[:],
                                start=(fc == 0), stop=(fc == 4),
                                skip_group_check=True)
                            nc.scalar.copy(qkv_raw[:, fc, tsl],
                                           accs[fc][:])
                        else:
                            nc.scalar.copy(vT_sb[:, tsl], accs[fc][:])
                    # v transpose for this chunk's 4 key blocks (Pool
                    # drains the psum so DVE stays free)
                    for sci in range(4):
                        scc = tcc * 4 + sci
                        vt_ps = psSq.tile([P, P], BF16, name="vt_ps",
                                          tag="sq")
                        nc.tensor.transpose(
                            vt_ps[:], vT_sb[:, scc * P:(scc + 1) * P],
                            ident_sb[:])
                        nc.scalar.copy(v_sb[:, scc, :], vt_ps[:])
                    # rms factors for this token chunk:
                    # rfac = 1/sqrt(ssq/128 + eps)
                    std = sqp.tile([5, 512], F32, name="std")
                    nc.scalar.activation(std[:], ssq_ps[:], AF.Sqrt,
                                         scale=1.0 / HEAD_DIM,
                                         bias=eps_sb[0:5, :])
                    nc.vector.reciprocal_approx_fast(rfac[:, tsl], std[:])
                    nc.gpsimd.dma_start(rfac_dr[:, tsl], rfac[:, tsl])
                    # normalize this token chunk; k and q0 finish their
                    # rope inline (rots borrow the psSq ring)
                    rope_chunk_inline(4, tcc, psSq, "sq")
                    rope_chunk_inline(0, tcc, psSq, "sq")
                    for fc in (1, 2, 3):
                        stt_chunk(fc, tcc)
                # q1's rope also completes here so D-block 1 never waits
                rope_rots(1, psSq, "sq")
                rope_finish(1)

            # ---------------- Stages C+D+E interleaved -------------------
            # Rope of q head h+1 is emitted at the END of D-block (pair0,
            # h) so the in-order PE queue never stalls on it; the first
            # half of stage E rides inside pair-1's D-blocks (PE slack
            # under the exp-bound phase).
            p3 = tc.alloc_tile_pool(name="p3", bufs=1)   # lives D..E
            atn_raw = [p3.tile([P, GQ, 1024], BF16, name=f"atn_raw{i}")
                       for i in range(2)]                         # 16KB
            atn_n = [p3.tile([P, GQ, 1024], BF16, name=f"atn_n{i}")
                     for i in range(2)]                           # 16KB
            woT_sb = p3.tile([P, GQ, DIM], BF16, name="woT_sb")   # 16KB
            dn_sb = [p3.tile([4, 1024], F32, name=f"dn_sb{i}")
                     for i in range(2)]
            nc.sync.dma_start(woT_sb[:], woT_d)  # prefetch (used in E)

            accp = tc.alloc_tile_pool(name="accp", bufs=2)
            ptp = tc.alloc_tile_pool(name="ptp", bufs=10)
            sdp = tc.alloc_tile_pool(name="sdp", bufs=1)
            sep = tc.alloc_tile_pool(name="sep", bufs=3)
            psS = tc.alloc_tile_pool(name="psS", bufs=2, space="PSUM")
            psPV = tc.alloc_tile_pool(name="psPV", bufs=1, space="PSUM")
            psE = tc.alloc_tile_pool(name="psE", bufs=2, space="PSUM")

            def d_block(pair, h, tail_rope=None):
                po = pair * 1024
                pv_ps = psPV.tile([P, 1024], F32, name="pv_ps")
                acc = accp.tile([P, 1024], BF16, name="acc")
                for scc in range(SC):
                    sp = psS.tile([P, 1024], F32, name="sp", tag="sp")
                    ksl = qk_sb[4][:, scc * P:(scc + 1) * P]
                    for q in range(2):
                        qs = slice(q * 512, (q + 1) * 512)
                        nc.tensor.matmul(
                            sp[:, qs], ksl,
                            qk_sb[h][:, po + q * 512:po + (q + 1) * 512],
                            start=True, stop=True)
                    pt = ptp.tile([P, 1024], BF16, name="pt")
                    nc.scalar.activation(pt[:], sp[:], AF.Exp,
                                         bias=zero_sb[:])
                    for q in range(2):
                        qs = slice(q * 512, (q + 1) * 512)
                        nc.tensor.matmul(pv_ps[:, qs], v_sb[:, scc, :],
                                         pt[:, qs], start=(scc == 0),
                                         stop=(scc == SC - 1))
                    if scc == 0:
                        nc.vector.tensor_copy(acc[:], pt[:])
                    else:
                        nc.vector.tensor_add(acc[:], acc[:], pt[:])
                # denominator partition-reduction, accumulated on SBUF
                for half in range(2):
                    hs = slice(half * 512, (half + 1) * 512)
                    dnt = psS.tile([4, 512], F32, name="dnt", tag="sp")
                    nc.tensor.matmul(dnt[:], esel_sb[:, h, 0:4],
                                     acc[:, hs], start=True, stop=True)
                    if h == 0:
                        nc.vector.tensor_copy(dn_sb[pair][:, hs], dnt[:])
                    else:
                        nc.vector.tensor_add(dn_sb[pair][:, hs],
                                             dn_sb[pair][:, hs], dnt[:])
                if tail_rope is not None:
                    rope_rots(tail_rope, psS, "sp")
                nc.vector.tensor_copy(atn_raw[pair][:, h, :], pv_ps[:])

            def d_norm(pair):
                """Reciprocal denominators + normalize pair's attn."""
                rd = sdp.tile([4, 1024], F32, name="rd", tag="rd")
                nc.vector.reciprocal_approx_fast(rd[:], dn_sb[pair][:])
                rdb = sdp.tile([4, 1024], BF16, name="rdb", tag="rdb")
                nc.vector.tensor_copy(rdb[:], rd[:])
                nc.gpsimd.dma_start(rd_dr[pair][:], rdb[:])
                for h in range(GQ):
                    rbh = sdp.tile([P, 1024], BF16, name="rbh", tag="rbh",
                                   bufs=2)
                    nc.gpsimd.dma_start(
                        rbh[:],
                        rd_dr[pair][h:h + 1, :].to_broadcast((P, 1024)))
                    nc.vector.tensor_mul(atn_n[pair][:, h, :],
                                         atn_raw[pair][:, h, :], rbh[:])

            def e_chunk(tcc, on_act=False):
                """Output projection for one 128-token chunk."""
                pr = tcc // 8
                tloc = (tcc % 8) * P
                ob = sep.tile([P, DIM], BF16, name="ob", tag="ob")
                for oc in range(4):
                    ps = psE.tile([P, 512], F32, name="out_ps", tag="eps")
                    for h in range(GQ):
                        nc.tensor.matmul(
                            ps[:], atn_n[pr][:, h, tloc:tloc + P],
                            woT_sb[:, h, oc * 512:(oc + 1) * 512],
                            start=(h == 0), stop=(h == GQ - 1))
                    osl = slice(oc * 512, (oc + 1) * 512)
                    if on_act:
                        nc.scalar.copy(ob[:, osl], ps[:])
                    else:
                        nc.vector.tensor_copy(ob[:, osl], ps[:])
                nc.sync.dma_start(out_d[tcc], ob[:])

            for pair in range(2):
                for h in range(GQ):
                    # rots for head h+2 land at the END of block h, so
                    # the finishing ops run a full block ahead of use
                    tail = h + 2 if (pair == 0 and h < GQ - 2) else None
                    d_block(pair, h, tail_rope=tail)
                    if tail is not None:
                        rope_finish(tail)
                    if pair == 1:
                        # first half of E rides in pair-1's PE slack
                        e_chunk(2 * h)
                        e_chunk(2 * h + 1)
                d_norm(pair)
                if pair == 0:
                    p1.release()   # qkv_raw/rfac/qn done after rope 3

            for tcc in range(8, SC):
                e_chunk(tcc, on_act=True)   # Act engine is idle post-D

            sep.release()
            sdp.release()
            ptp.release()
            accp.release()
            p3.release()
            ropep.release()
            psE.release()
            psPV.release()
            psS.release()
            p2.release()

    nc.compile()
    return nc


def make_in_maps(x, wqkv, wo, q_norm_w, k_norm_w, freqs_cos, freqs_sin):
    """Build the 8 per-core input maps. Core c = b*4 + g."""
    bf = ml_dtypes.bfloat16
    x = np.asarray(x, np.float32)
    wqkv = np.asarray(wqkv, np.float32)
    wo = np.asarray(wo, np.float32)
    q_norm_w = np.asarray(q_norm_w, np.float32)
    k_norm_w = np.asarray(k_norm_w, np.float32)
    cosT = np.ascontiguousarray(
        np.asarray(freqs_cos, np.float32)[:, 0, :].T).astype(bf)
    sinT = np.ascontiguousarray(
        np.asarray(freqs_sin, np.float32)[:, 0, :].T).astype(bf)

    normw = np.empty((P, 2), np.float32)
    normw[:, 0] = q_norm_w * np.float32(1.0 / np.sqrt(HEAD_DIM))
    normw[:, 1] = k_norm_w

    prot = np.zeros((P, P), np.float32)
    prot[np.arange(1, P, 2), np.arange(0, P, 2)] = -1.0
    prot[np.arange(0, P, 2), np.arange(1, P, 2)] = 1.0
    prot = prot.astype(bf)
    ident = np.eye(P, dtype=np.float32).astype(bf)
    esel = np.zeros((P, 5, 5), np.float32)
    for c in range(5):
        esel[:, c, c] = 1.0
    esel = esel.astype(bf)

    q_size = N_HEADS * HEAD_DIM
    kv_size = N_KV * HEAD_DIM
    in_maps = []
    for b in range(B):
        # [tc, p, kc, u]: xT[kc*128+p, tc*512+u] pre-tiled for DMA locality
        xT = np.ascontiguousarray(
            x[b].reshape(TC, 512, KC, P).transpose(0, 3, 2, 1))
        for g in range(N_KV):
            wq = wqkv[g * GF:(g + 1) * GF]
            wk = wqkv[q_size + g * HEAD_DIM:q_size + (g + 1) * HEAD_DIM]
            wv = wqkv[q_size + kv_size + g * HEAD_DIM:
                      q_size + kv_size + (g + 1) * HEAD_DIM]
            wqkvT = np.ascontiguousarray(
                np.concatenate([wq, wk, wv], axis=0).T
                .reshape(KC, P, NF).transpose(1, 0, 2))
            woT = np.ascontiguousarray(
                wo[:, g * GF:(g + 1) * GF].T.reshape(GQ, HEAD_DIM, DIM)
                .transpose(1, 0, 2)).astype(bf)
            in_maps.append({
                "xT": xT, "wqkvT": wqkvT, "woT": woT,
                "cosT": cosT, "sinT": sinT, "normw": normw,
                "prot": prot, "ident": ident, "esel": esel,
            })
    return in_maps


def run(in_maps, trace=False):
    global _CACHED_NC
    if _CACHED_NC is None:
        _CACHED_NC = build_nc()
    return bass_utils.run_bass_kernel_spmd(
        _CACHED_NC, in_maps, core_ids=list(range(8)), trace=trace)


def kernel(x, wqkv, wo, q_norm_w, k_norm_w, freqs_cos, freqs_sin):
    in_maps = make_in_maps(x, wqkv, wo, q_norm_w, k_norm_w,
                           freqs_cos, freqs_sin)
    res = run(in_maps, trace=False)
    out = np.zeros((B, S, DIM), np.float32)
    for b in range(B):
        for g in range(N_KV):
            o = res.results[b * N_KV + g]["out"]    # [SC, P, DIM] bf16
            out[b] += np.asarray(o, np.float32).reshape(S, DIM)
    return out


# revision 46
# speedup vs baseline: 1.0474x; 1.0474x over previous
"""Trainium2 Bass kernel for nn_Attention_3710851743764.

Full attention block: qkv proj -> per-head RMSNorm(q,k) -> RoPE -> GQA
attention (16 q heads, 4 kv heads, S=2048, D=128) -> out proj.

Sharding: 8 cores = 2 (batch) x 4 (kv-head groups). Each core computes its
batch's qkv for its group (4 q heads + 1 kv head), full attention for those
heads, and a partial output projection (its 512 wo columns); the host sums
the 4 partials per batch.

Projection runs in f32r (full PE rate, no input quantization); attention
data (q/k/v, probabilities, outputs) is bf16 with f32 PSUM accumulation.
Dataflow is fully "transposed" (features on partitions, tokens on free):
  qkvT[f,t]   = mm(lhsT=wqkvT[d,f], rhs=xT[d,t])   kc-outer: 6 psum accums
  ssq[c,t]    = mm(lhsT=esel[:,c,:], rhs=square(qkvT_c))     (RMS factors)
  qn          = qkv * normw * rfac  (stt on DVE, emitted inside stage A)
  rot         = mm(lhsT=P_rot, rhs=qn)   (k and q0 in A's tail, q1-3
                tucked into D-block tails so the in-order PE never stalls)
  scoresT[s,t]= mm(lhsT=kT[:,s-blk], rhs=qT_h)    per 128-s block
  pT          = exp(scoresT)       (no max subtraction: |score|<=sqrt(128))
  attnT[d,t]  = mm(lhsT=v[s-blk,d], rhs=pT)       accumulated over s
  denom       = DVE bf16 accumulation of pT over s-blocks + tiny esel
                matmuls, accumulated on SBUF (frees PSUM for E interleave)
  out[t,o]    = mm(lhsT=attnT_n[f,t-blk], rhs=woT[f,o]); the first half
                of E runs inside pair-1's attention (PE slack under exp)
"""

import sys

sys.path.insert(0, "/opt/trn_rl_repo")

import numpy as np
import ml_dtypes

import concourse.bass as bass
import concourse.tile as tile
from concourse import bacc, mybir
from concourse import bass_utils

F32 = mybir.dt.float32
F32R = mybir.dt.float32r
BF16 = mybir.dt.bfloat16
AF = mybir.ActivationFunctionType
OP = mybir.AluOpType

DIM = 2048
N_HEADS = 16
N_KV = 4
HEAD_DIM = 128
B = 2
S = 2048
EPS = float(np.finfo(np.float32).eps)
GQ = N_HEADS // N_KV          # q heads per group = 4
GF = GQ * HEAD_DIM            # group q features = 512
P = 128
KC = DIM // P                 # 16 contraction chunks for projections
TC = 4                        # token chunks of 512
SC = S // P                   # 16 key chunks of 128
NF = GF + 2 * HEAD_DIM        # 768 qkv features per group
FC = NF // P                  # 6 feature chunks

_CACHED_NC = None


def build_nc():
    """Build the single-core Bass program (same program for all 8 cores)."""
    nc = bacc.Bacc("TRN2", target_bir_lowering=False, debug=False,
                   num_devices=8)

    xT_d = nc.dram_tensor("xT", [TC, P, KC, 512], F32R,
                          kind="ExternalInput").ap()
    wqkvT_d = nc.dram_tensor("wqkvT", [P, KC, NF], F32R,
                             kind="ExternalInput").ap()
    woT_d = nc.dram_tensor("woT", [HEAD_DIM, GQ, DIM], BF16,
                           kind="ExternalInput").ap()
    cosT_d = nc.dram_tensor("cosT", [HEAD_DIM, S], BF16,
                            kind="ExternalInput").ap()
    sinT_d = nc.dram_tensor("sinT", [HEAD_DIM, S], BF16,
                            kind="ExternalInput").ap()
    normw_d = nc.dram_tensor("normw", [P, 2], F32, kind="ExternalInput").ap()
    prot_d = nc.dram_tensor("prot", [P, P], BF16, kind="ExternalInput").ap()
    ident_d = nc.dram_tensor("ident", [P, P], BF16, kind="ExternalInput").ap()
    esel_d = nc.dram_tensor("esel", [P, 5, 5], BF16,
                            kind="ExternalInput").ap()
    out_d = nc.dram_tensor("out", [SC, P, DIM], BF16,
                           kind="ExternalOutput").ap()

    with tile.TileContext(nc) as tc:
        with (
            tc.tile_pool(name="consts", bufs=1) as cp,
        ):
            dramp = tc.alloc_tile_pool(name="dram_scratch", bufs=1,
                                       space="DRAM")
            rfac_dr = dramp.tile([5, S], F32, name="rfac_dr")
            rd_dr = [dramp.tile([4, 1024], BF16, name=f"rd_dr{i}")
                     for i in range(2)]
            # right-side stack: p2 (lives A..D) below p1 (lives A..mid-D)
            p2 = tc.alloc_tile_pool(name="p2", bufs=1, side="right")
            qk_sb = [p2.tile([P, S], BF16, name=f"qk_sb{i}")
                     for i in range(5)]                           # 20KB
            v_sb = p2.tile([P, SC, HEAD_DIM], BF16, name="v_sb")  # 4KB
            p1 = tc.alloc_tile_pool(name="p1", bufs=1, side="right")
            qkv_raw = p1.tile([P, 5, S], BF16, name="qkv_raw")    # 20KB
            vT_sb = p1.tile([P, S], BF16, name="vT_sb")           # 4KB
            rfac = p1.tile([5, S], F32, name="rfac")
            qn_sb = [p1.tile([P, S], BF16, name=f"qn_sb{i}")
                     for i in range(5)]                           # 20KB

            cos_sb = cp.tile([HEAD_DIM, S], BF16, name="cos_sb")
            sin_sb = cp.tile([HEAD_DIM, S], BF16, name="sin_sb")
            normw_sb = cp.tile([P, 2], F32, name="normw_sb")
            prot_sb = cp.tile([P, P], BF16, name="prot_sb")
            ident_sb = cp.tile([P, P], BF16, name="ident_sb")
            esel_sb = cp.tile([P, 5, 5], BF16, name="esel_sb")
            eps_sb = cp.tile([P, 1], F32, name="eps_sb")
            zero_sb = cp.tile([P, 1], F32, name="zero_sb")
            nc.vector.memset(eps_sb[:], EPS)
            nc.vector.memset(zero_sb[:], 0.0)
            # consts via the gpsimd queue; sync queue feeds stage A
            nc.gpsimd.dma_start(esel_sb[:], esel_d)
            nc.gpsimd.dma_start(ident_sb[:], ident_d)
            nc.gpsimd.dma_start(prot_sb[:], prot_d)
            nc.gpsimd.dma_start(normw_sb[:], normw_d)
            nc.gpsimd.dma_start(cos_sb[:], cosT_d)
            nc.gpsimd.dma_start(sin_sb[:], sinT_d)

            ropep = tc.alloc_tile_pool(name="ropep", bufs=1)

            def stt_chunk(fc, tcc):
                """qn = qkv * normw * rfac for one (feature, token) chunk."""
                tsl = slice(tcc * 512, (tcc + 1) * 512)
                wcol = 0 if fc < 4 else 1
                rb = ropep.tile([P, 512], F32, name="rb", tag="rb", bufs=3)
                nc.gpsimd.dma_start(
                    rb[:],
                    rfac_dr[fc:fc + 1, tsl].to_broadcast((P, 512)))
                nc.vector.scalar_tensor_tensor(
                    qn_sb[fc][:, tsl], qkv_raw[:, fc, tsl],
                    normw_sb[:, wcol:wcol + 1], rb[:],
                    op0=OP.mult, op1=OP.mult)

            rot_ps = {}

            def rope_rots(fc, pool, tag):
                """The four [P,512] rotation matmuls for one feature row."""
                rot_ps[fc] = []
                for tcc in range(TC):
                    rp = pool.tile([P, 512], F32, name="rot_ps", tag=tag)
                    nc.tensor.matmul(
                        rp[:], prot_sb[:],
                        qn_sb[fc][:, tcc * 512:(tcc + 1) * 512],
                        start=True, stop=True)
                    rot_ps[fc].append(rp)

            def rope_finish_chunk(fc, tcc, rp):
                """qk = qn*cos (Pool) + rot*sin (DVE), added in place."""
                tsl = slice(tcc * 512, (tcc + 1) * 512)
                nc.gpsimd.tensor_mul(qk_sb[fc][:, tsl],
                                     qn_sb[fc][:, tsl], cos_sb[:, tsl])
                rs = ropep.tile([P, 512], BF16, name="rs", tag="rs",
                                bufs=3)
                nc.vector.tensor_mul(rs[:], rp[:], sin_sb[:, tsl])
                nc.vector.tensor_add(qk_sb[fc][:, tsl],
                                     qk_sb[fc][:, tsl], rs[:])

            def rope_finish(fc):
                for tcc in range(TC):
                    rope_finish_chunk(fc, tcc, rot_ps[fc][tcc])

            def rope_chunk_inline(fc, tcc, pool, tag):
                """stt + rot + finish for one (feature, token) chunk."""
                stt_chunk(fc, tcc)
                rp = pool.tile([P, 512], F32, name="rot_ps", tag=tag)
                nc.tensor.matmul(
                    rp[:], prot_sb[:],
                    qn_sb[fc][:, tcc * 512:(tcc + 1) * 512],
                    start=True, stop=True)
                rope_finish_chunk(fc, tcc, rp)

            # ---------------- Stage A: qkv projection + squares ----------
            # kc-outer so the first matmuls only need the first x/w chunks
            # (fast DMA ramp); 6 psum accumulators live per token chunk.
            # The rmsnorm stt chunks ride along at each token-chunk tail,
            # and the k/q0 ropes run at the very end (rots via psSq ring).
            with (
                tc.tile_pool(name="stA", bufs=4) as sa,
                tc.tile_pool(name="sqp", bufs=2) as sqp,
                tc.tile_pool(name="wq_pool", bufs=1) as wp,
                tc.tile_pool(name="psA", bufs=1, space="PSUM") as psA,
                tc.tile_pool(name="psSq", bufs=2, space="PSUM") as psSq,
            ):
                wq_sb = wp.tile([P, KC, NF], F32R, name="wq_sb")  # 48KB
                for tcc in range(TC):
                    xts = []
                    for kc4 in range(0, KC, 4):
                        if tcc == 0:
                            nc.sync.dma_start(wq_sb[:, kc4:kc4 + 4, :],
                                              wqkvT_d[:, kc4:kc4 + 4, :])
                        xtc = sa.tile([P, 4, 512], F32R, name="xt",
                                      tag="xt")
                        nc.sync.dma_start(xtc[:],
                                          xT_d[tcc, :, kc4:kc4 + 4, :])
                        xts.append(xtc)
                    accs = [psA.tile([P, 512], F32, name=f"acc{fc}")
                            for fc in range(FC)]
                    for kc in range(KC):
                        for fc in range(FC):
                            nc.tensor.matmul(
                                accs[fc][:],
                                wq_sb[:, kc, fc * P:(fc + 1) * P],
                                xts[kc // 4][:, kc % 4, :],
                                start=(kc == 0), stop=(kc == KC - 1))
                    tsl = slice(tcc * 512, (tcc + 1) * 512)
                    ssq_ps = psSq.tile([5, 512], F32, name="ssq_ps",
                                       tag="sq")
                    for fc in range(FC):
                        if fc < 5:
                            sq = sqp.tile([P, 512], BF16, name="sq")
                            nc.scalar.activation(sq[:], accs[fc][:],
                                                 AF.Square, bias=zero_sb[:])
                            nc.tensor.matmul(
                                ssq_ps[:], esel_sb[:, fc, 0:5], sq[:],
                                start=(fc == 0), stop=(fc == 4),
                                skip_group_check=True)
                            nc.scalar.copy(qkv_raw[:, fc, tsl],
                                           accs[fc][:])
                        else:
                            nc.scalar.copy(vT_sb[:, tsl], accs[fc][:])
                    # v transpose for this chunk's 4 key blocks (Pool
                    # drains the psum so DVE stays free)
                    for sci in range(4):
                        scc = tcc * 4 + sci
                        vt_ps = psSq.tile([P, P], BF16, name="vt_ps",
                                          tag="sq")
                        nc.tensor.transpose(
                            vt_ps[:], vT_sb[:, scc * P:(scc + 1) * P],
                            ident_sb[:])
                        nc.scalar.copy(v_sb[:, scc, :], vt_ps[:])
                    # rms factors for this token chunk:
                    # rfac = 1/sqrt(ssq/128 + eps)
                    std = sqp.tile([5, 512], F32, name="std")
                    nc.scalar.activation(std[:], ssq_ps[:], AF.Sqrt,
                                         scale=1.0 / HEAD_DIM,
                                         bias=eps_sb[0:5, :])
                    nc.vector.reciprocal_approx_fast(rfac[:, tsl], std[:])
                    nc.gpsimd.dma_start(rfac_dr[:, tsl], rfac[:, tsl])
                    # normalize this token chunk; k and q0 finish their
                    # rope inline (rots borrow the psSq ring)
                    rope_chunk_inline(4, tcc, psSq, "sq")
                    rope_chunk_inline(0, tcc, psSq, "sq")
                    for fc in (1, 2, 3):
                        stt_chunk(fc, tcc)
                # q1's rope also completes here so D-block 1 never waits
                rope_rots(1, psSq, "sq")
                rope_finish(1)

            # ---------------- Stages C+D+E interleaved -------------------
            # Rope of q head h+1 is emitted at the END of D-block (pair0,
            # h) so the in-order PE queue never stalls on it; the first
            # half of stage E rides inside pair-1's D-blocks (PE slack
            # under the exp-bound phase).
            p3 = tc.alloc_tile_pool(name="p3", bufs=1)   # lives D..E
            atn_raw = [p3.tile([P, GQ, 1024], BF16, name=f"atn_raw{i}")
                       for i in range(2)]                         # 16KB
            atn_n = [p3.tile([P, GQ, 1024], BF16, name=f"atn_n{i}")
                     for i in range(2)]                           # 16KB
            woT_sb = p3.tile([P, GQ, DIM], BF16, name="woT_sb")   # 16KB
            dn_sb = [p3.tile([4, 1024], F32, name=f"dn_sb{i}")
                     for i in range(2)]
            nc.sync.dma_start(woT_sb[:], woT_d)  # prefetch (used in E)

            accp = tc.alloc_tile_pool(name="accp", bufs=2)
            ptp = tc.alloc_tile_pool(name="ptp", bufs=10)
            sdp = tc.alloc_tile_pool(name="sdp", bufs=1)
            sep = tc.alloc_tile_pool(name="sep", bufs=3)
            psS = tc.alloc_tile_pool(name="psS", bufs=2, space="PSUM")
            psPV = tc.alloc_tile_pool(name="psPV", bufs=1, space="PSUM")
            psE = tc.alloc_tile_pool(name="psE", bufs=2, space="PSUM")

            def d_block(pair, h, tail_rope=None):
                po = pair * 1024
                pv_ps = psPV.tile([P, 1024], F32, name="pv_ps")
                acc = accp.tile([P, 1024], BF16, name="acc")
                for scc in range(SC):
                    sp = psS.tile([P, 1024], F32, name="sp", tag="sp")
                    ksl = qk_sb[4][:, scc * P:(scc + 1) * P]
                    for q in range(2):
                        qs = slice(q * 512, (q + 1) * 512)
                        nc.tensor.matmul(
                            sp[:, qs], ksl,
                            qk_sb[h][:, po + q * 512:po + (q + 1) * 512],
                            start=True, stop=True)
                    pt = ptp.tile([P, 1024], BF16, name="pt")
                    nc.scalar.activation(pt[:], sp[:], AF.Exp,
                                         bias=zero_sb[:])
                    for q in range(2):
                        qs = slice(q * 512, (q + 1) * 512)
                        nc.tensor.matmul(pv_ps[:, qs], v_sb[:, scc, :],
                                         pt[:, qs], start=(scc == 0),
                                         stop=(scc == SC - 1))
                    if scc == 0:
                        nc.vector.tensor_copy(acc[:], pt[:])
                    else:
                        nc.vector.tensor_add(acc[:], acc[:], pt[:])
                # denominator partition-reduction, accumulated on SBUF.
                # dnt/rot tiles use the psE ring (idle during pair 0) so
                # the next block's scores never wait on their readers.
                for half in range(2):
                    hs = slice(half * 512, (half + 1) * 512)
                    dnt = psE.tile([4, 512], F32, name="dnt", tag="eps")
                    nc.tensor.matmul(dnt[:], esel_sb[:, h, 0:4],
                                     acc[:, hs], start=True, stop=True)
                    if h == 0:
                        nc.vector.tensor_copy(dn_sb[pair][:, hs], dnt[:])
                    else:
                        nc.vector.tensor_add(dn_sb[pair][:, hs],
                                             dn_sb[pair][:, hs], dnt[:])
                if tail_rope is not None:
                    rope_rots(tail_rope, psE, "eps")
                nc.vector.tensor_copy(atn_raw[pair][:, h, :], pv_ps[:])

            def d_norm(pair):
                """Reciprocal denominators + normalize pair's attn."""
                rd = sdp.tile([4, 1024], F32, name="rd", tag="rd")
                nc.vector.reciprocal_approx_fast(rd[:], dn_sb[pair][:])
                rdb = sdp.tile([4, 1024], BF16, name="rdb", tag="rdb")
                nc.vector.tensor_copy(rdb[:], rd[:])
                nc.gpsimd.dma_start(rd_dr[pair][:], rdb[:])
                for h in range(GQ):
                    rbh = sdp.tile([P, 1024], BF16, name="rbh", tag="rbh",
                                   bufs=2)
                    nc.gpsimd.dma_start(
                        rbh[:],
                        rd_dr[pair][h:h + 1, :].to_broadcast((P, 1024)))
                    nc.vector.tensor_mul(atn_n[pair][:, h, :],
                                         atn_raw[pair][:, h, :], rbh[:])

            def e_chunk(tcc, on_act=False):
                """Output projection for one 128-token chunk."""
                pr = tcc // 8
                tloc = (tcc % 8) * P
                ob = sep.tile([P, DIM], BF16, name="ob", tag="ob")
                for oc in range(4):
                    ps = psE.tile([P, 512], F32, name="out_ps", tag="eps")
                    for h in range(GQ):
                        nc.tensor.matmul(
                            ps[:], atn_n[pr][:, h, tloc:tloc + P],
                            woT_sb[:, h, oc * 512:(oc + 1) * 512],
                            start=(h == 0), stop=(h == GQ - 1))
                    osl = slice(oc * 512, (oc + 1) * 512)
                    if on_act:
                        nc.scalar.copy(ob[:, osl], ps[:])
                    else:
                        nc.vector.tensor_copy(ob[:, osl], ps[:])
                nc.sync.dma_start(out_d[tcc], ob[:])

            for pair in range(2):
                for h in range(GQ):
                    # rots for head h+2 land at the END of block h, so
                    # the finishing ops run a full block ahead of use
                    tail = h + 2 if (pair == 0 and h < GQ - 2) else None
                    d_block(pair, h, tail_rope=tail)
                    if tail is not None:
                        rope_finish(tail)
                    if pair == 1:
                        # first half of E rides in pair-1's PE slack
                        e_chunk(2 * h)
                        e_chunk(2 * h + 1)
                d_norm(pair)
                if pair == 0:
                    p1.release()   # qkv_raw/rfac/qn done after rope 3

            for tcc in range(8, SC):
                e_chunk(tcc, on_act=True)   # Act engine is idle post-D

            sep.release()
            sdp.release()
            ptp.release()
            accp.release()
            p3.release()
            ropep.release()
            psE.release()
            psPV.release()
            psS.release()
            p2.release()

    nc.compile()
    return nc


def make_in_maps(x, wqkv, wo, q_norm_w, k_norm_w, freqs_cos, freqs_sin):
    """Build the 8 per-core input maps. Core c = b*4 + g."""
    bf = ml_dtypes.bfloat16
    x = np.asarray(x, np.float32)
    wqkv = np.asarray(wqkv, np.float32)
    wo = np.asarray(wo, np.float32)
    q_norm_w = np.asarray(q_norm_w, np.float32)
    k_norm_w = np.asarray(k_norm_w, np.float32)
    cosT = np.ascontiguousarray(
        np.asarray(freqs_cos, np.float32)[:, 0, :].T).astype(bf)
    sinT = np.ascontiguousarray(
        np.asarray(freqs_sin, np.float32)[:, 0, :].T).astype(bf)

    normw = np.empty((P, 2), np.float32)
    normw[:, 0] = q_norm_w * np.float32(1.0 / np.sqrt(HEAD_DIM))
    normw[:, 1] = k_norm_w

    prot = np.zeros((P, P), np.float32)
    prot[np.arange(1, P, 2), np.arange(0, P, 2)] = -1.0
    prot[np.arange(0, P, 2), np.arange(1, P, 2)] = 1.0
    prot = prot.astype(bf)
    ident = np.eye(P, dtype=np.float32).astype(bf)
    esel = np.zeros((P, 5, 5), np.float32)
    for c in range(5):
        esel[:, c, c] = 1.0
    esel = esel.astype(bf)

    q_size = N_HEADS * HEAD_DIM
    kv_size = N_KV * HEAD_DIM
    in_maps = []
    for b in range(B):
        # [tc, p, kc, u]: xT[kc*128+p, tc*512+u] pre-tiled for DMA locality
        xT = np.ascontiguousarray(
            x[b].reshape(TC, 512, KC, P).transpose(0, 3, 2, 1))
        for g in range(N_KV):
            wq = wqkv[g * GF:(g + 1) * GF]
            wk = wqkv[q_size + g * HEAD_DIM:q_size + (g + 1) * HEAD_DIM]
            wv = wqkv[q_size + kv_size + g * HEAD_DIM:
                      q_size + kv_size + (g + 1) * HEAD_DIM]
            wqkvT = np.ascontiguousarray(
                np.concatenate([wq, wk, wv], axis=0).T
                .reshape(KC, P, NF).transpose(1, 0, 2))
            woT = np.ascontiguousarray(
                wo[:, g * GF:(g + 1) * GF].T.reshape(GQ, HEAD_DIM, DIM)
                .transpose(1, 0, 2)).astype(bf)
            in_maps.append({
                "xT": xT, "wqkvT": wqkvT, "woT": woT,
                "cosT": cosT, "sinT": sinT, "normw": normw,
                "prot": prot, "ident": ident, "esel": esel,
            })
    return in_maps


def run(in_maps, trace=False):
    global _CACHED_NC
    if _CACHED_NC is None:
        _CACHED_NC = build_nc()
    return bass_utils.run_bass_kernel_spmd(
        _CACHED_NC, in_maps, core_ids=list(range(8)), trace=trace)


def kernel(x, wqkv, wo, q_norm_w, k_norm_w, freqs_cos, freqs_sin):
    in_maps = make_in_maps(x, wqkv, wo, q_norm_w, k_norm_w,
                           freqs_cos, freqs_sin)
    res = run(in_maps, trace=False)
    out = np.zeros((B, S, DIM), np.float32)
    for b in range(B):
        for g in range(N_KV):
            o = res.results[b * N_KV + g]["out"]    # [SC, P, DIM] bf16
            out[b] += np.asarray(o, np.float32).reshape(S, DIM)
    return out


# revision 48
# speedup vs baseline: 1.0551x; 1.0074x over previous
"""Trainium2 Bass kernel for nn_Attention_3710851743764.

Full attention block: qkv proj -> per-head RMSNorm(q,k) -> RoPE -> GQA
attention (16 q heads, 4 kv heads, S=2048, D=128) -> out proj.

Sharding: 8 cores = 2 (batch) x 4 (kv-head groups). Each core computes its
batch's qkv for its group (4 q heads + 1 kv head), full attention for those
heads, and a partial output projection (its 512 wo columns); the host sums
the 4 partials per batch.

Projection runs in f32r (full PE rate, no input quantization); attention
data (q/k/v, probabilities, outputs) is bf16 with f32 PSUM accumulation.
Dataflow is fully "transposed" (features on partitions, tokens on free):
  qkvT[f,t]   = mm(lhsT=wqkvT[d,f], rhs=xT[d,t])   kc-outer: 6 psum accums
  ssq[c,t]    = mm(lhsT=esel[:,c,:], rhs=square(qkvT_c))     (RMS factors)
  qn          = qkv * normw * rfac  (stt on DVE, emitted inside stage A)
  rot         = mm(lhsT=P_rot, rhs=qn)   (k and q0 in A's tail, q1-3
                tucked into D-block tails so the in-order PE never stalls)
  scoresT[s,t]= mm(lhsT=kT[:,s-blk], rhs=qT_h)    per 128-s block
  pT          = exp(scoresT)       (no max subtraction: |score|<=sqrt(128))
  attnT[d,t]  = mm(lhsT=v[s-blk,d], rhs=pT)       accumulated over s
  denom       = DVE bf16 accumulation of pT over s-blocks + tiny esel
                matmuls, accumulated on SBUF (frees PSUM for E interleave)
  out[t,o]    = mm(lhsT=attnT_n[f,t-blk], rhs=woT[f,o]); the first half
                of E runs inside pair-1's attention (PE slack under exp)
"""

import sys

sys.path.insert(0, "/opt/trn_rl_repo")

import numpy as np
import ml_dtypes

import concourse.bass as bass
import concourse.tile as tile
from concourse import bacc, mybir
from concourse import bass_utils

F32 = mybir.dt.float32
F32R = mybir.dt.float32r
BF16 = mybir.dt.bfloat16
AF = mybir.ActivationFunctionType
OP = mybir.AluOpType

DIM = 2048
N_HEADS = 16
N_KV = 4
HEAD_DIM = 128
B = 2
S = 2048
EPS = float(np.finfo(np.float32).eps)
GQ = N_HEADS // N_KV          # q heads per group = 4
GF = GQ * HEAD_DIM            # group q features = 512
P = 128
KC = DIM // P                 # 16 contraction chunks for projections
TC = 4                        # token chunks of 512
SC = S // P                   # 16 key chunks of 128
NF = GF + 2 * HEAD_DIM        # 768 qkv features per group
FC = NF // P                  # 6 feature chunks

_CACHED_NC = None


def build_nc():
    """Build the single-core Bass program (same program for all 8 cores)."""
    nc = bacc.Bacc("TRN2", target_bir_lowering=False, debug=False,
                   num_devices=8)

    xT_d = nc.dram_tensor("xT", [TC, P, KC, 512], F32R,
                          kind="ExternalInput").ap()
    wqkvT_d = nc.dram_tensor("wqkvT", [P, KC, NF], F32R,
                             kind="ExternalInput").ap()
    woT_d = nc.dram_tensor("woT", [HEAD_DIM, GQ, DIM], BF16,
                           kind="ExternalInput").ap()
    cosT_d = nc.dram_tensor("cosT", [HEAD_DIM, S], BF16,
                            kind="ExternalInput").ap()
    sinT_d = nc.dram_tensor("sinT", [HEAD_DIM, S], BF16,
                            kind="ExternalInput").ap()
    normw_d = nc.dram_tensor("normw", [P, 2], F32, kind="ExternalInput").ap()
    prot_d = nc.dram_tensor("prot", [P, P], BF16, kind="ExternalInput").ap()
    ident_d = nc.dram_tensor("ident", [P, P], BF16, kind="ExternalInput").ap()
    esel_d = nc.dram_tensor("esel", [P, 5, 5], BF16,
                            kind="ExternalInput").ap()
    out_d = nc.dram_tensor("out", [SC, P, DIM], BF16,
                           kind="ExternalOutput").ap()

    with tile.TileContext(nc) as tc:
        with (
            tc.tile_pool(name="consts", bufs=1) as cp,
        ):
            dramp = tc.alloc_tile_pool(name="dram_scratch", bufs=1,
                                       space="DRAM")
            rfac_dr = dramp.tile([5, S], F32, name="rfac_dr")
            rd_dr = [dramp.tile([4, 1024], BF16, name=f"rd_dr{i}")
                     for i in range(2)]
            # right-side stack: p2 (lives A..D) below p1 (lives A..mid-D)
            p2 = tc.alloc_tile_pool(name="p2", bufs=1, side="right")
            qk_sb = [p2.tile([P, S], BF16, name=f"qk_sb{i}")
                     for i in range(5)]                           # 20KB
            v_sb = p2.tile([P, SC, HEAD_DIM], BF16, name="v_sb")  # 4KB
            p1 = tc.alloc_tile_pool(name="p1", bufs=1, side="right")
            qkv_raw = p1.tile([P, 5, S], BF16, name="qkv_raw")    # 20KB
            vT_sb = p1.tile([P, S], BF16, name="vT_sb")           # 4KB
            rfac = p1.tile([5, S], F32, name="rfac")
            qn_sb = [p1.tile([P, S], BF16, name=f"qn_sb{i}")
                     for i in range(5)]                           # 20KB

            cos_sb = cp.tile([HEAD_DIM, S], BF16, name="cos_sb")
            sin_sb = cp.tile([HEAD_DIM, S], BF16, name="sin_sb")
            normw_sb = cp.tile([P, 2], F32, name="normw_sb")
            prot_sb = cp.tile([P, P], BF16, name="prot_sb")
            ident_sb = cp.tile([P, P], BF16, name="ident_sb")
            esel_sb = cp.tile([P, 5, 5], BF16, name="esel_sb")
            eps_sb = cp.tile([P, 1], F32, name="eps_sb")
            zero_sb = cp.tile([P, 1], F32, name="zero_sb")
            nc.vector.memset(eps_sb[:], EPS)
            nc.vector.memset(zero_sb[:], 0.0)
            # consts via the gpsimd queue; sync queue feeds stage A
            nc.gpsimd.dma_start(esel_sb[:], esel_d)
            nc.gpsimd.dma_start(ident_sb[:], ident_d)
            nc.gpsimd.dma_start(prot_sb[:], prot_d)
            nc.gpsimd.dma_start(normw_sb[:], normw_d)
            nc.gpsimd.dma_start(cos_sb[:], cosT_d)
            nc.gpsimd.dma_start(sin_sb[:], sinT_d)

            ropep = tc.alloc_tile_pool(name="ropep", bufs=1)

            def stt_chunk(fc, tcc):
                """qn = qkv * normw * rfac for one (feature, token) chunk."""
                tsl = slice(tcc * 512, (tcc + 1) * 512)
                wcol = 0 if fc < 4 else 1
                rb = ropep.tile([P, 512], F32, name="rb", tag="rb", bufs=3)
                nc.gpsimd.dma_start(
                    rb[:],
                    rfac_dr[fc:fc + 1, tsl].to_broadcast((P, 512)))
                nc.vector.scalar_tensor_tensor(
                    qn_sb[fc][:, tsl], qkv_raw[:, fc, tsl],
                    normw_sb[:, wcol:wcol + 1], rb[:],
                    op0=OP.mult, op1=OP.mult)

            rot_ps = {}

            def rope_rots(fc, pool, tag):
                """The four [P,512] rotation matmuls for one feature row."""
                rot_ps[fc] = []
                for tcc in range(TC):
                    rp = pool.tile([P, 512], F32, name="rot_ps", tag=tag)
                    nc.tensor.matmul(
                        rp[:], prot_sb[:],
                        qn_sb[fc][:, tcc * 512:(tcc + 1) * 512],
                        start=True, stop=True)
                    rot_ps[fc].append(rp)

            def rope_finish_chunk(fc, tcc, rp):
                """qk = qn*cos (Pool) + rot*sin (DVE), added in place."""
                tsl = slice(tcc * 512, (tcc + 1) * 512)
                nc.gpsimd.tensor_mul(qk_sb[fc][:, tsl],
                                     qn_sb[fc][:, tsl], cos_sb[:, tsl])
                rs = ropep.tile([P, 512], BF16, name="rs", tag="rs",
                                bufs=3)
                nc.vector.tensor_mul(rs[:], rp[:], sin_sb[:, tsl])
                nc.vector.tensor_add(qk_sb[fc][:, tsl],
                                     qk_sb[fc][:, tsl], rs[:])

            def rope_finish(fc):
                for tcc in range(TC):
                    rope_finish_chunk(fc, tcc, rot_ps[fc][tcc])

            def rope_chunk_inline(fc, tcc, pool, tag):
                """stt + rot + finish for one (feature, token) chunk."""
                stt_chunk(fc, tcc)
                rp = pool.tile([P, 512], F32, name="rot_ps", tag=tag)
                nc.tensor.matmul(
                    rp[:], prot_sb[:],
                    qn_sb[fc][:, tcc * 512:(tcc + 1) * 512],
                    start=True, stop=True)
                rope_finish_chunk(fc, tcc, rp)

            # ---------------- Stage A: qkv projection + squares ----------
            # kc-outer so the first matmuls only need the first x/w chunks
            # (fast DMA ramp); 6 psum accumulators live per token chunk.
            # The rmsnorm stt chunks ride along at each token-chunk tail,
            # and the k/q0 ropes run at the very end (rots via psSq ring).
            with (
                tc.tile_pool(name="stA", bufs=4) as sa,
                tc.tile_pool(name="sqp", bufs=2) as sqp,
                tc.tile_pool(name="wq_pool", bufs=1) as wp,
                tc.tile_pool(name="psA", bufs=1, space="PSUM") as psA,
                tc.tile_pool(name="psSq", bufs=2, space="PSUM") as psSq,
            ):
                wq_sb = wp.tile([P, KC, NF], F32R, name="wq_sb")  # 48KB
                for tcc in range(TC):
                    xts = []
                    for kc4 in range(0, KC, 4):
                        if tcc == 0:
                            # Act queue is idle here: wq and xt loads
                            # stream on two queues in parallel at the ramp
                            nc.scalar.dma_start(wq_sb[:, kc4:kc4 + 4, :],
                                                wqkvT_d[:, kc4:kc4 + 4, :])
                        xtc = sa.tile([P, 4, 512], F32R, name="xt",
                                      tag="xt")
                        nc.sync.dma_start(xtc[:],
                                          xT_d[tcc, :, kc4:kc4 + 4, :])
                        xts.append(xtc)
                    accs = [psA.tile([P, 512], F32, name=f"acc{fc}")
                            for fc in range(FC)]
                    for kc in range(KC):
                        for fc in range(FC):
                            nc.tensor.matmul(
                                accs[fc][:],
                                wq_sb[:, kc, fc * P:(fc + 1) * P],
                                xts[kc // 4][:, kc % 4, :],
                                start=(kc == 0), stop=(kc == KC - 1))
                    tsl = slice(tcc * 512, (tcc + 1) * 512)
                    ssq_ps = psSq.tile([5, 512], F32, name="ssq_ps",
                                       tag="sq")
                    for fc in range(FC):
                        if fc < 5:
                            sq = sqp.tile([P, 512], BF16, name="sq")
                            nc.scalar.activation(sq[:], accs[fc][:],
                                                 AF.Square, bias=zero_sb[:])
                            nc.tensor.matmul(
                                ssq_ps[:], esel_sb[:, fc, 0:5], sq[:],
                                start=(fc == 0), stop=(fc == 4),
                                skip_group_check=True)
                            nc.scalar.copy(qkv_raw[:, fc, tsl],
                                           accs[fc][:])
                        else:
                            nc.scalar.copy(vT_sb[:, tsl], accs[fc][:])
                    # v transpose for this chunk's 4 key blocks (Pool
                    # drains the psum so DVE stays free)
                    for sci in range(4):
                        scc = tcc * 4 + sci
                        vt_ps = psSq.tile([P, P], BF16, name="vt_ps",
                                          tag="sq")
                        nc.tensor.transpose(
                            vt_ps[:], vT_sb[:, scc * P:(scc + 1) * P],
                            ident_sb[:])
                        nc.scalar.copy(v_sb[:, scc, :], vt_ps[:])
                    # rms factors for this token chunk:
                    # rfac = 1/sqrt(ssq/128 + eps)
                    std = sqp.tile([5, 512], F32, name="std")
                    nc.scalar.activation(std[:], ssq_ps[:], AF.Sqrt,
                                         scale=1.0 / HEAD_DIM,
                                         bias=eps_sb[0:5, :])
                    nc.vector.reciprocal_approx_fast(rfac[:, tsl], std[:])
                    nc.gpsimd.dma_start(rfac_dr[:, tsl], rfac[:, tsl])
                    # normalize this token chunk; k and q0 finish their
                    # rope inline (rots borrow the psSq ring)
                    rope_chunk_inline(4, tcc, psSq, "sq")
                    rope_chunk_inline(0, tcc, psSq, "sq")
                    for fc in (1, 2, 3):
                        stt_chunk(fc, tcc)
                # q1's rope also completes here so D-block 1 never waits
                rope_rots(1, psSq, "sq")
                rope_finish(1)

            # ---------------- Stages C+D+E interleaved -------------------
            # Rope of q head h+1 is emitted at the END of D-block (pair0,
            # h) so the in-order PE queue never stalls on it; the first
            # half of stage E rides inside pair-1's D-blocks (PE slack
            # under the exp-bound phase).
            p3 = tc.alloc_tile_pool(name="p3", bufs=1)   # lives D..E
            atn_raw = [p3.tile([P, GQ, 1024], BF16, name=f"atn_raw{i}")
                       for i in range(2)]                         # 16KB
            atn_n = [p3.tile([P, GQ, 1024], BF16, name=f"atn_n{i}")
                     for i in range(2)]                           # 16KB
            woT_sb = p3.tile([P, GQ, DIM], BF16, name="woT_sb")   # 16KB
            dn_sb = [p3.tile([4, 1024], F32, name=f"dn_sb{i}")
                     for i in range(2)]
            nc.sync.dma_start(woT_sb[:], woT_d)  # prefetch (used in E)

            accp = tc.alloc_tile_pool(name="accp", bufs=2)
            ptp = tc.alloc_tile_pool(name="ptp", bufs=12)
            sdp = tc.alloc_tile_pool(name="sdp", bufs=1)
            sep = tc.alloc_tile_pool(name="sep", bufs=3)
            psS = tc.alloc_tile_pool(name="psS", bufs=2, space="PSUM")
            psPV = tc.alloc_tile_pool(name="psPV", bufs=1, space="PSUM")
            psE = tc.alloc_tile_pool(name="psE", bufs=2, space="PSUM")

            def d_block(pair, h, tail_rope=None):
                po = pair * 1024
                pv_ps = psPV.tile([P, 1024], F32, name="pv_ps")
                acc = accp.tile([P, 1024], BF16, name="acc")
                for scc in range(SC):
                    sp = psS.tile([P, 1024], F32, name="sp", tag="sp")
                    ksl = qk_sb[4][:, scc * P:(scc + 1) * P]
                    for q in range(2):
                        qs = slice(q * 512, (q + 1) * 512)
                        nc.tensor.matmul(
                            sp[:, qs], ksl,
                            qk_sb[h][:, po + q * 512:po + (q + 1) * 512],
                            start=True, stop=True)
                    pt = ptp.tile([P, 1024], BF16, name="pt")
                    nc.scalar.activation(pt[:], sp[:], AF.Exp,
                                         bias=zero_sb[:])
                    for q in range(2):
                        qs = slice(q * 512, (q + 1) * 512)
                        nc.tensor.matmul(pv_ps[:, qs], v_sb[:, scc, :],
                                         pt[:, qs], start=(scc == 0),
                                         stop=(scc == SC - 1))
                    if scc == 0:
                        nc.vector.tensor_copy(acc[:], pt[:])
                    else:
                        nc.vector.tensor_add(acc[:], acc[:], pt[:])
                # denominator partition-reduction, accumulated on SBUF.
                # dnt/rot tiles use the psE ring (idle during pair 0) so
                # the next block's scores never wait on their readers.
                for half in range(2):
                    hs = slice(half * 512, (half + 1) * 512)
                    dnt = psE.tile([4, 512], F32, name="dnt", tag="eps")
                    nc.tensor.matmul(dnt[:], esel_sb[:, h, 0:4],
                                     acc[:, hs], start=True, stop=True)
                    if h == 0:
                        nc.vector.tensor_copy(dn_sb[pair][:, hs], dnt[:])
                    else:
                        nc.vector.tensor_add(dn_sb[pair][:, hs],
                                             dn_sb[pair][:, hs], dnt[:])
                if tail_rope is not None:
                    rope_rots(tail_rope, psE, "eps")
                nc.vector.tensor_copy(atn_raw[pair][:, h, :], pv_ps[:])

            def d_norm(pair):
                """Reciprocal denominators + normalize pair's attn."""
                rd = sdp.tile([4, 1024], F32, name="rd", tag="rd")
                nc.vector.reciprocal_approx_fast(rd[:], dn_sb[pair][:])
                rdb = sdp.tile([4, 1024], BF16, name="rdb", tag="rdb")
                nc.vector.tensor_copy(rdb[:], rd[:])
                nc.gpsimd.dma_start(rd_dr[pair][:], rdb[:])
                for h in range(GQ):
                    rbh = sdp.tile([P, 1024], BF16, name="rbh", tag="rbh",
                                   bufs=2)
                    nc.gpsimd.dma_start(
                        rbh[:],
                        rd_dr[pair][h:h + 1, :].to_broadcast((P, 1024)))
                    nc.vector.tensor_mul(atn_n[pair][:, h, :],
                                         atn_raw[pair][:, h, :], rbh[:])

            def e_chunk(tcc, on_act=False):
                """Output projection for one 128-token chunk."""
                pr = tcc // 8
                tloc = (tcc % 8) * P
                ob = sep.tile([P, DIM], BF16, name="ob", tag="ob")
                for oc in range(4):
                    ps = psE.tile([P, 512], F32, name="out_ps", tag="eps")
                    for h in range(GQ):
                        nc.tensor.matmul(
                            ps[:], atn_n[pr][:, h, tloc:tloc + P],
                            woT_sb[:, h, oc * 512:(oc + 1) * 512],
                            start=(h == 0), stop=(h == GQ - 1))
                    osl = slice(oc * 512, (oc + 1) * 512)
                    if on_act:
                        nc.scalar.copy(ob[:, osl], ps[:])
                    else:
                        nc.vector.tensor_copy(ob[:, osl], ps[:])
                nc.sync.dma_start(out_d[tcc], ob[:])

            for pair in range(2):
                for h in range(GQ):
                    # rots for head h+2 land at the END of block h, so
                    # the finishing ops run a full block ahead of use
                    tail = h + 2 if (pair == 0 and h < GQ - 2) else None
                    d_block(pair, h, tail_rope=tail)
                    if tail is not None:
                        rope_finish(tail)
                    if pair == 1:
                        # first half of E rides in pair-1's PE slack
                        e_chunk(2 * h)
                        e_chunk(2 * h + 1)
                d_norm(pair)
                if pair == 0:
                    p1.release()   # qkv_raw/rfac/qn done after rope 3

            for tcc in range(8, SC):
                e_chunk(tcc, on_act=True)   # Act engine is idle post-D

            sep.release()
            sdp.release()
            ptp.release()
            accp.release()
            p3.release()
            ropep.release()
            psE.release()
            psPV.release()
            psS.release()
            p2.release()

    nc.compile()
    return nc


def make_in_maps(x, wqkv, wo, q_norm_w, k_norm_w, freqs_cos, freqs_sin):
    """Build the 8 per-core input maps. Core c = b*4 + g."""
    bf = ml_dtypes.bfloat16
    x = np.asarray(x, np.float32)
    wqkv = np.asarray(wqkv, np.float32)
    wo = np.asarray(wo, np.float32)
    q_norm_w = np.asarray(q_norm_w, np.float32)
    k_norm_w = np.asarray(k_norm_w, np.float32)
    cosT = np.ascontiguousarray(
        np.asarray(freqs_cos, np.float32)[:, 0, :].T).astype(bf)
    sinT = np.ascontiguousarray(
        np.asarray(freqs_sin, np.float32)[:, 0, :].T).astype(bf)

    normw = np.empty((P, 2), np.float32)
    normw[:, 0] = q_norm_w * np.float32(1.0 / np.sqrt(HEAD_DIM))
    normw[:, 1] = k_norm_w

    prot = np.zeros((P, P), np.float32)
    prot[np.arange(1, P, 2), np.arange(0, P, 2)] = -1.0
    prot[np.arange(0, P, 2), np.arange(1, P, 2)] = 1.0
    prot = prot.astype(bf)
    ident = np.eye(P, dtype=np.float32).astype(bf)
    esel = np.zeros((P, 5, 5), np.float32)
    for c in range(5):
        esel[:, c, c] = 1.0
    esel = esel.astype(bf)

    q_size = N_HEADS * HEAD_DIM
    kv_size = N_KV * HEAD_DIM
    in_maps = []
    for b in range(B):
        # [tc, p, kc, u]: xT[kc*128+p, tc*512+u] pre-tiled for DMA locality
        xT = np.ascontiguousarray(
            x[b].reshape(TC, 512, KC, P).transpose(0, 3, 2, 1))
        for g in range(N_KV):
            wq = wqkv[g * GF:(g + 1) * GF]
            wk = wqkv[q_size + g * HEAD_DIM:q_size + (g + 1) * HEAD_DIM]
            wv = wqkv[q_size + kv_size + g * HEAD_DIM:
                      q_size + kv_size + (g + 1) * HEAD_DIM]
            wqkvT = np.ascontiguousarray(
                np.concatenate([wq, wk, wv], axis=0).T
                .reshape(KC, P, NF).transpose(1, 0, 2))
            woT = np.ascontiguousarray(
                wo[:, g * GF:(g + 1) * GF].T.reshape(GQ, HEAD_DIM, DIM)
                .transpose(1, 0, 2)).astype(bf)
            in_maps.append({
                "xT": xT, "wqkvT": wqkvT, "woT": woT,
                "cosT": cosT, "sinT": sinT, "normw": normw,
                "prot": prot, "ident": ident, "esel": esel,
            })
    return in_maps


def run(in_maps, trace=False):
    global _CACHED_NC
    if _CACHED_NC is None:
        _CACHED_NC = build_nc()
    return bass_utils.run_bass_kernel_spmd(
        _CACHED_NC, in_maps, core_ids=list(range(8)), trace=trace)


def kernel(x, wqkv, wo, q_norm_w, k_norm_w, freqs_cos, freqs_sin):
    in_maps = make_in_maps(x, wqkv, wo, q_norm_w, k_norm_w,
                           freqs_cos, freqs_sin)
    res = run(in_maps, trace=False)
    out = np.zeros((B, S, DIM), np.float32)
    for b in range(B):
        for g in range(N_KV):
            o = res.results[b * N_KV + g]["out"]    # [SC, P, DIM] bf16
            out[b] += np.asarray(o, np.float32).reshape(S, DIM)
    return out
